# revision 22
# baseline (speedup 1.0000x reference)
"""FM pairwise-interaction layer on 8 Trainium2 NeuronCores (fp16 pipeline).

out[b, p] = x[b, I1[p]] * x[b, I2[p]] * dot(w[I1[p]], w[I2[p]])   for all
P = 512*511/2 = 130816 strict upper-triangle pairs, batch 1024.

Strategy (data-parallel over batch, 128 rows per core):
  *  wdot = W @ W.T has rank 4, so a K=4 fp16 matmul per j1-block computes
        psum[b, c] = sum_k (x[b,j1] w[j1,k]) * (w[j2,k])  =  x[b,j1]*wdot[p]
     with x.T and WP[k,p] = w[I2,k] shipped fp16 from the host.
  *  Blocks are processed in QUADS (4 j1-blocks -> one 4-bank PSUM tile,
     uniform even width w = ceil_even(n0)); merged consumer ops amortize
     per-op init overhead.  Output is fp16 (rel err ~1e-3 << 2e-2 gate);
     host upcasts, halving the HBM write roofline vs f32.
  *  Per-quad consumer classes (tunable split):
       A: DVE tensor_mul direct from PSUM (1x mode)
       B: ACT evacuates PSUM->SBUF fp16, DVE tensor_mul at 2x mode
          (both operands fp16, step 1, 4B-aligned via parity copies of x)
       C: ACT evacuation + GpSimd tensor_mul (shares a SBUF port with
          DVE 2-port ops, so kept small)
     balancing DVE/ACT/GPSIMD below the fp16 DMA write roofline.
"""

import numpy as np

import concourse.bass as bass
import concourse.mybir as mybir
from concourse import bacc
from concourse.tile import TileContext
import concourse.bass_utils as bass_utils

NF = 512          # features
K = 4             # latent dim
B = 1024          # batch
NCORES = 8
BS = B // NCORES  # 128 batch rows per core
P = NF * (NF - 1) // 2  # 130816 pairs

F16 = mybir.dt.float16
_f16np = np.float16

QUAD = 2          # blocks per PSUM tile unit (parity-strided)
J_SPLIT = 260     # j1 < J_SPLIT -> class B/C (ACT-assisted), else class A
C_EVERY = 0      # every C_EVERY-th B/C quad goes to GPSIMD (0 = disable)
CH_MAX = 6144     # stage tile columns (fp16); flush threshold
XPAD = 8
STAGE_BUFS = 3
TMP_BUFS = 8
PSUM_BUFS = 4     # [128, 512*QUAD] tiles; QUAD*PSUM_BUFS <= 8 banks
REPS = 1
DMA_ONLY = 0      # 1: skip all compute, only DMA flushes (timing experiments)
NO_DMA = 0        # 1: skip output flush DMAs (timing experiments)
SKIP_CONSUME = 0  # 1: matmuls only, no consumer ops (timing experiments)
SKIP_MULT = 0     # 1: matmuls + ACT evacs only, no DVE/GPS mults (timing)
TRACE = False
LAST_RESULT = {}
_last_in_maps = None

_WPAD = 8


def _off(j1):
    return j1 * (NF - 1) - j1 * (j1 - 1) // 2


_GOFF = [_off(0), _off(128), _off(256), _off(384), P]
_GW = [_GOFF[g + 1] - _GOFF[g] for g in range(4)]  # 57280, 40896, 24512, 8128


def _quads():
    """Processing order: round-robin across the 4 j1-groups so consecutive
    matmuls land on different PE row-groups (tile_position concurrency).
    Each unit is QUAD same-parity blocks (j1, j1+2, ..) from one group, so
    the class-B multiply is a single 2x-mode DVE op (xs cols j1+1, j1+3, ..
    share parity -> 4B-aligned outer step 2).  Units come in low/high parity
    couples covering 2*QUAD consecutive blocks."""
    def ew(j1):
        n = NF - 1 - j1
        return n + (n & 1)

    cursors = [128 * g for g in range(4)]
    limits = [min(128 * (g + 1), NF - 1) for g in range(4)]
    pend = [None] * 4   # deferred odd-parity unit per group
    out = []
    while True:
        alive = False
        for g in range(4):
            if pend[g] is not None:
                out.append(pend[g])
                pend[g] = None
                alive = True
                continue
            j1 = cursors[g]
            if j1 >= limits[g]:
                continue
            alive = True
            rem = limits[g] - j1
            if rem >= 2 * QUAD:
                out.append((j1, QUAD, ew(j1), 2))
                pend[g] = (j1 + 1, QUAD, ew(j1 + 1), 2)
                cursors[g] += 2 * QUAD
            elif rem >= 2:
                ne = (rem + 1) // 2
                no = rem // 2
                out.append((j1, ne, ew(j1), 2))
                if no:
                    pend[g] = (j1 + 1, no, ew(j1 + 1), 2)
                cursors[g] += rem
            else:
                out.append((j1, 1, ew(j1), 1))
                cursors[g] += 1
        if not alive:
            break
    return out


def _relayout():
    """Recompute processing order / stage layout from current knobs."""
    global _QUADS, _POFF, P_PAD, _CHUNKS
    _QUADS = _quads()
    _POFF = {}
    pp = 0
    for j1s, nb, w, js in _QUADS:
        for c in range(nb):
            _POFF[j1s + c * js] = pp
            pp += w
    P_PAD = pp
    _CHUNKS = []
    acc = 0
    for j1s, nb, w, js in _QUADS:
        if acc + nb * w > CH_MAX:
            _CHUNKS.append(acc)
            acc = 0
        acc += nb * w
    if acc:
        _CHUNKS.append(acc)
    assert sum(_CHUNKS) == P_PAD


_relayout()


def ap2d(sliced, dims):
    """Copy of AP `sliced` with its free dims replaced by [step, count] pairs."""
    c = sliced.copy()
    v = c.ap
    part = [list(v[0])]
    while len(v) > 0:
        v.pop()
    for d in part + [list(x) for x in dims]:
        v.append(d)
    c.ap = v
    return c


A_EVERY = 0       # among B-region quads, every A_EVERY-th becomes class A


def _quad_class(qi, j1):
    if j1 >= J_SPLIT:
        return "A"
    if C_EVERY and qi % C_EVERY == C_EVERY - 1:
        return "C"
    if A_EVERY and qi % A_EVERY == A_EVERY - 1:
        return "A"
    return "B"


def _build_nc():
    nc = bacc.Bacc("TRN2", target_bir_lowering=False, debug=False,
                   num_devices=NCORES)

    xs_d = nc.dram_tensor("xs2", (2, BS, NF + XPAD), F16,
                          kind="ExternalInput").ap()
    xt_d = nc.dram_tensor("xt4", (4, K, 128 * 128), F16,
                          kind="ExternalInput").ap()
    wp_d = [nc.dram_tensor(f"wp{g}", (K, _GW[g] + _WPAD), F16,
                           kind="ExternalInput").ap() for g in range(4)]
    out_d = nc.dram_tensor("out", (BS * P_PAD,), F16, kind="ExternalOutput").ap()

    with TileContext(nc) as tc:
        with tc.tile_pool(name="sb", bufs=1) as sb, \
             tc.tile_pool(name="stg", bufs=STAGE_BUFS) as stg, \
             tc.tile_pool(name="tp", bufs=TMP_BUFS) as tp, \
             tc.tile_pool(name="ps", bufs=PSUM_BUFS, space="PSUM") as ps:

            xe = sb.tile([128, NF + XPAD], F16, tag="xe")
            xo = sb.tile([128, NF + XPAD], F16, tag="xo")
            nc.sync.dma_start(out=xe[:, :], in_=xs_d[0])
            nc.sync.dma_start(out=xo[:, :], in_=xs_d[1])

            xt = sb.tile([128, 128 * 128], F16, tag="xt")
            wp = sb.tile([128, _GW[0] + _WPAD], F16, tag="wp")
            for g in range(4):
                nc.sync.dma_start(out=xt[32 * g:32 * g + K, :], in_=xt_d[g])
                nc.sync.dma_start(out=wp[32 * g:32 * g + K, 0:_GW[g] + _WPAD],
                                  in_=wp_d[g][:])

            def lhs(j1):
                g = j1 // 128
                r = j1 - 128 * g
                return xt[32 * g:32 * g + K, r * 128:(r + 1) * 128]

            def rhs(j1, n):
                g = j1 // 128
                lo = _off(j1) - _GOFF[g]
                return wp[32 * g:32 * g + K, lo:lo + n]

            def xsl_a(col, cnt, step, n):
                # alignment-free slice (1x-mode consumers)
                return ap2d(xe[:, col:col + 1], [[step, cnt], [1, n]])

            def xsl_b(col, cnt, step, n):
                # 4B-aligned fp16 slice via parity copies (2x-mode consumers)
                t, c0 = (xe, col) if col % 2 == 0 else (xo, col - 1)
                return ap2d(t[:, c0:c0 + 1], [[step, cnt], [1, n]])

            if DMA_ONLY:
                zstage = sb.tile([128, CH_MAX], F16, tag="zstage")
                nc.vector.memset(zstage[:, :], 0.0)
                for _rep in range(REPS):
                    s = 0
                    for u in _CHUNKS:
                        dst = out_d[s * 128:(s + u) * 128]
                        dst = dst.rearrange("(p f) -> p f", p=128)
                        nc.sync.dma_start(out=dst, in_=zstage[:, 0:u])
                        s += u
            else:
                for _rep in range(REPS):
                    main_pass(nc, stg, tp, ps, xsl_a, xsl_b, lhs, rhs, out_d)

    nc.compile()
    return nc


def main_pass(nc, stg, tp, ps, xsl_a, xsl_b, lhs, rhs, out_d):
    used = 0
    chunk_base = 0
    stage = stg.tile([128, CH_MAX], F16, tag="stage")

    def flush():
        nonlocal used, chunk_base, stage
        if used == 0:
            return
        if not NO_DMA:
            dst = out_d[chunk_base * 128:(chunk_base + used) * 128]
            dst = dst.rearrange("(p f) -> p f", p=128)
            nc.sync.dma_start(out=dst, in_=stage[:, 0:used])
        chunk_base += used
        used = 0
        if chunk_base < P_PAD:
            stage = stg.tile([128, CH_MAX], F16, tag="stage")

    for qi, (j1s, nb, w, js) in enumerate(_QUADS):
        if used + nb * w > CH_MAX:
            flush()
        if DMA_ONLY:
            used += nb * w
            continue
        cls = _quad_class(qi, j1s)
        psum = ps.tile([128, 512 * QUAD], mybir.dt.float32, tag="psum")
        for c in range(nb):
            nc.tensor.matmul(psum[:, 512 * c:512 * c + w],
                             lhs(j1s + c * js), rhs(j1s + c * js, w),
                             start=True, stop=True,
                             tile_position=(32 * ((j1s + c * js) // 128), 0))
        O = used
        if SKIP_CONSUME:
            used += nb * w
            continue
        if cls == "A":
            nc.vector.tensor_mul(
                out=ap2d(stage[:, O:O + 1], [[w, nb], [1, w]]),
                in0=ap2d(psum[:, 0:1], [[512, nb], [1, w]]),
                in1=xsl_a(j1s + 1, nb, js, w))
        else:
            tmp = tp.tile([128, 512 * QUAD], F16, tag="tmp")
            nc.scalar.copy(
                out=ap2d(tmp[:, 0:1], [[512, nb], [1, w]]),
                in_=ap2d(psum[:, 0:1], [[512, nb], [1, w]]))
            if SKIP_MULT:
                used += nb * w
                continue
            if cls == "B" and js == 2:
                # same-parity pair: one 2x-mode op covers both blocks
                nc.vector.tensor_mul(
                    out=ap2d(stage[:, O:O + 1], [[w, nb], [1, w]]),
                    in0=ap2d(tmp[:, 0:1], [[512, nb], [1, w]]),
                    in1=xsl_b(j1s + 1, nb, 2, w))
            elif cls == "B":
                ne = (nb + 1) // 2   # even c: 0, 2
                no = nb // 2         # odd  c: 1, 3
                nc.vector.tensor_mul(
                    out=ap2d(stage[:, O:O + 1], [[2 * w, ne], [1, w]]),
                    in0=ap2d(tmp[:, 0:1], [[1024, ne], [1, w]]),
                    in1=xsl_b(j1s + 1, ne, 2, w))
                if no:
                    nc.vector.tensor_mul(
                        out=ap2d(stage[:, O + w:O + w + 1],
                                 [[2 * w, no], [1, w]]),
                        in0=ap2d(tmp[:, 512:513], [[1024, no], [1, w]]),
                        in1=xsl_b(j1s + 2, no, 2, w))
            else:  # C
                nc.gpsimd.tensor_mul(
                    out=ap2d(stage[:, O:O + 1], [[w, nb], [1, w]]),
                    in0=ap2d(tmp[:, 0:1], [[512, nb], [1, w]]),
                    in1=xsl_a(j1s + 1, nb, js, w))
        used += nb * w
    flush()


_NC_CACHE = None


def kernel(x, weight):
    global _NC_CACHE, LAST_RESULT, _last_in_maps
    x = np.ascontiguousarray(x, dtype=np.float32)
    weight = np.ascontiguousarray(weight, dtype=np.float32)
    assert x.shape == (B, NF) and weight.shape == (NF, K)

    # ---- host-side weight-derived constants (fp16)
    i1, i2 = np.triu_indices(NF, k=1)
    wp_full = weight[i2].T.astype(_f16np)                # [K, P] = w[j2, k]
    wp_in = {}
    for g in range(4):
        arr = np.zeros((K, _GW[g] + _WPAD), dtype=_f16np)
        arr[:, 0:_GW[g]] = wp_full[:, _GOFF[g]:_GOFF[g + 1]]
        wp_in[f"wp{g}"] = arr

    # ---- per-core inputs
    in_maps = []
    for c in range(NCORES):
        xc = x[c * BS:(c + 1) * BS]                      # [128, 512] f32
        xh = xc.astype(_f16np)
        xs2 = np.zeros((2, BS, NF + XPAD), dtype=_f16np)
        xs2[0, :, 0:NF] = xh
        xs2[1, :, 0:NF - 1] = xh[:, 1:]
        xct = xc.T                                        # [512, 128] f32
        xt4 = np.empty((4, K, 128 * 128), dtype=_f16np)
        for k in range(K):
            # lhsT[k, :, b] = x[b, j1] * w[j1, k], j1-major rows
            zk = (xct * weight[:, k:k + 1]).astype(_f16np)  # [512, 128]
        for g in range(4):
            for k in range(K):
                zk = (xct[128 * g:128 * (g + 1)]
                      * weight[128 * g:128 * (g + 1), k:k + 1]).astype(_f16np)
                xt4[g, k] = zk.reshape(-1)
        m = {"xs2": xs2, "xt4": xt4}
        m.update(wp_in)
        in_maps.append(m)

    _last_in_maps = in_maps
    if _NC_CACHE is None:
        _NC_CACHE = _build_nc()
    nc = _NC_CACHE

    res = bass_utils.run_bass_kernel_spmd(nc, in_maps,
                                          core_ids=list(range(NCORES)),
                                          trace=TRACE)
    LAST_RESULT = {"exec_time_ns": res.exec_time_ns,
                   "trace": res.instructions_and_trace}

    # ---- host unpack: de-chunk, de-pad, upcast
    idx = np.empty(P, dtype=np.int64)
    for j1, (po, n) in enumerate(
            (_POFF[j], NF - 1 - j) for j in range(NF - 1)):
        idx[_off(j1):_off(j1) + n] = np.arange(po, po + n)
    cores = []
    for r in res.results:
        flat = r["out"].reshape(-1)
        b2 = np.empty((BS, P_PAD), dtype=_f16np)
        s = 0
        for u in _CHUNKS:
            b2[:, s:s + u] = flat[128 * s:128 * (s + u)].reshape(BS, u)
            s += u
        cores.append(b2)
    blob_all = np.concatenate(cores, axis=0)              # [1024, P_PAD]
    out = blob_all[:, idx].astype(np.float32)
    return out


# revision 24
# speedup vs baseline: 1.3453x; 1.3453x over previous
"""FM pairwise-interaction layer on 8 Trainium2 NeuronCores (fp16 pipeline).

out[b, p] = x[b, I1[p]] * x[b, I2[p]] * dot(w[I1[p]], w[I2[p]])   for all
P = 512*511/2 = 130816 strict upper-triangle pairs, batch 1024.

Strategy (data-parallel over batch, 128 rows per core):
  *  wdot = W @ W.T has rank 4, so a K=4 fp16 matmul per j1-block computes
        psum[b, c] = sum_k (x[b,j1] w[j1,k]) * (w[j2,k])  =  x[b,j1]*wdot[p]
     with x.T and WP[k,p] = w[I2,k] shipped fp16 from the host.
  *  Output is fp16 (rel err ~7e-4 << 2e-2 gate); host upcasts.  This
     halves the HBM write roofline vs f32 (33.6 MB/core, ~103 us at the
     measured ~330 GB/s per-core DMA write bandwidth).
  *  The PE on this part runs at 1.2 GHz (no HAM ramp observed), so serial
     matmuls are ~112 us.  Fix: blocks are processed round-robin across the
     four 128-row j1-groups; their lhsT/rhs data live at partition offsets
     32g, so consecutive matmuls hit different PE row-groups
     (tile_position) and execute concurrently (measured 82 us, 4 PSUM
     tiles in flight).
  *  Each unit is a PAIR of same-parity blocks (j1, j1+2) -> one 2-bank
     PSUM tile, slot width w = ceil_even(n0); merged 2D-AP consumer ops
     amortize per-op init overhead, and the parity stride keeps fp16
     operands 4B-aligned for DVE 2x mode.
  *  Per-unit consumer classes (tunable split via J_SPLIT):
       A: DVE tensor_mul direct from PSUM (1x mode, no ACT involvement)
       B: ACT evacuates PSUM->SBUF fp16, then one DVE tensor_mul at 2x
          mode (all operands fp16, inner step 1, 4B-aligned)
     balancing DVE (~1.04 cyc/elem A-path) against ACT+DVE (~1.0+0.5).
     GPSIMD is not used: its SBUF port is locked out by every DVE
     tensor_tensor op (measured -20 us when enabled).
"""

import numpy as np

import concourse.bass as bass
import concourse.mybir as mybir
from concourse import bacc
from concourse.tile import TileContext
import concourse.bass_utils as bass_utils

NF = 512          # features
K = 4             # latent dim
B = 1024          # batch
NCORES = 8
BS = B // NCORES  # 128 batch rows per core
P = NF * (NF - 1) // 2  # 130816 pairs

F16 = mybir.dt.float16
_f16np = np.float16

QUAD = 2          # blocks per PSUM tile unit (parity-strided)
J_SPLIT = 260     # j1 < J_SPLIT -> class B/C (ACT-assisted), else class A
C_EVERY = 0      # every C_EVERY-th B/C quad goes to GPSIMD (0 = disable)
CH_MAX = 6144     # stage tile columns (fp16); flush threshold
XPAD = 8
STAGE_BUFS = 3
TMP_BUFS = 8
PSUM_BUFS = 4     # [128, 512*QUAD] tiles; QUAD*PSUM_BUFS <= 8 banks
REPS = 1
DMA_ONLY = 0      # 1: skip all compute, only DMA flushes (timing experiments)
NO_DMA = 0        # 1: skip output flush DMAs (timing experiments)
SKIP_CONSUME = 0  # 1: matmuls only, no consumer ops (timing experiments)
SKIP_MULT = 0     # 1: matmuls + ACT evacs only, no DVE/GPS mults (timing)
TRACE = False
LAST_RESULT = {}
_last_in_maps = None

_WPAD = 8


def _off(j1):
    return j1 * (NF - 1) - j1 * (j1 - 1) // 2


_GOFF = [_off(0), _off(128), _off(256), _off(384), P]
_GW = [_GOFF[g + 1] - _GOFF[g] for g in range(4)]  # 57280, 40896, 24512, 8128


def _quads():
    """Processing order: round-robin across the 4 j1-groups so consecutive
    matmuls land on different PE row-groups (tile_position concurrency).
    Each unit is QUAD same-parity blocks (j1, j1+2, ..) from one group, so
    the class-B multiply is a single 2x-mode DVE op (xs cols j1+1, j1+3, ..
    share parity -> 4B-aligned outer step 2).  Units come in low/high parity
    couples covering 2*QUAD consecutive blocks."""
    def ew(j1):
        n = NF - 1 - j1
        return n + (n & 1)

    cursors = [128 * g for g in range(4)]
    limits = [min(128 * (g + 1), NF - 1) for g in range(4)]
    pend = [None] * 4   # deferred odd-parity unit per group
    out = []
    while True:
        alive = False
        for g in range(4):
            if pend[g] is not None:
                out.append(pend[g])
                pend[g] = None
                alive = True
                continue
            j1 = cursors[g]
            if j1 >= limits[g]:
                continue
            alive = True
            rem = limits[g] - j1
            if rem >= 2 * QUAD:
                out.append((j1, QUAD, ew(j1), 2))
                pend[g] = (j1 + 1, QUAD, ew(j1 + 1), 2)
                cursors[g] += 2 * QUAD
            elif rem >= 2:
                ne = (rem + 1) // 2
                no = rem // 2
                out.append((j1, ne, ew(j1), 2))
                if no:
                    pend[g] = (j1 + 1, no, ew(j1 + 1), 2)
                cursors[g] += rem
            else:
                out.append((j1, 1, ew(j1), 1))
                cursors[g] += 1
        if not alive:
            break
    return out


def _relayout():
    """Recompute processing order / stage layout from current knobs."""
    global _QUADS, _POFF, P_PAD, _CHUNKS
    _QUADS = _quads()
    _POFF = {}
    pp = 0
    for j1s, nb, w, js in _QUADS:
        for c in range(nb):
            _POFF[j1s + c * js] = pp
            pp += w
    P_PAD = pp
    _CHUNKS = []
    acc = 0
    for j1s, nb, w, js in _QUADS:
        if acc + nb * w > CH_MAX:
            _CHUNKS.append(acc)
            acc = 0
        acc += nb * w
    if acc:
        _CHUNKS.append(acc)
    assert sum(_CHUNKS) == P_PAD


_relayout()


def ap2d(sliced, dims):
    """Copy of AP `sliced` with its free dims replaced by [step, count] pairs."""
    c = sliced.copy()
    v = c.ap
    part = [list(v[0])]
    while len(v) > 0:
        v.pop()
    for d in part + [list(x) for x in dims]:
        v.append(d)
    c.ap = v
    return c


A_EVERY = 0       # among B-region quads, every A_EVERY-th becomes class A


def _quad_class(qi, j1):
    if j1 >= J_SPLIT:
        return "A"
    if C_EVERY and qi % C_EVERY == C_EVERY - 1:
        return "C"
    if A_EVERY and qi % A_EVERY == A_EVERY - 1:
        return "A"
    return "B"


def _build_nc():
    nc = bacc.Bacc("TRN2", target_bir_lowering=False, debug=False,
                   num_devices=NCORES)

    xs_d = nc.dram_tensor("xs2", (2, BS, NF + XPAD), F16,
                          kind="ExternalInput").ap()
    xt_d = nc.dram_tensor("xt4", (4, K, 128 * 128), F16,
                          kind="ExternalInput").ap()
    wp_d = [nc.dram_tensor(f"wp{g}", (K, _GW[g] + _WPAD), F16,
                           kind="ExternalInput").ap() for g in range(4)]
    out_d = nc.dram_tensor("out", (BS * P_PAD,), F16, kind="ExternalOutput").ap()

    with TileContext(nc) as tc:
        with tc.tile_pool(name="sb", bufs=1) as sb, \
             tc.tile_pool(name="stg", bufs=STAGE_BUFS) as stg, \
             tc.tile_pool(name="tp", bufs=TMP_BUFS) as tp, \
             tc.tile_pool(name="ps", bufs=PSUM_BUFS, space="PSUM") as ps:

            xe = sb.tile([128, NF + XPAD], F16, tag="xe")
            xo = sb.tile([128, NF + XPAD], F16, tag="xo")
            nc.sync.dma_start(out=xe[:, :], in_=xs_d[0])
            nc.sync.dma_start(out=xo[:, :], in_=xs_d[1])

            xt = sb.tile([128, 128 * 128], F16, tag="xt")
            wp = sb.tile([128, _GW[0] + _WPAD], F16, tag="wp")
            for g in range(4):
                nc.sync.dma_start(out=xt[32 * g:32 * g + K, :], in_=xt_d[g])
                nc.sync.dma_start(out=wp[32 * g:32 * g + K, 0:_GW[g] + _WPAD],
                                  in_=wp_d[g][:])

            def lhs(j1):
                g = j1 // 128
                r = j1 - 128 * g
                return xt[32 * g:32 * g + K, r * 128:(r + 1) * 128]

            def rhs(j1, n):
                g = j1 // 128
                lo = _off(j1) - _GOFF[g]
                return wp[32 * g:32 * g + K, lo:lo + n]

            def xsl_a(col, cnt, step, n):
                # alignment-free slice (1x-mode consumers)
                return ap2d(xe[:, col:col + 1], [[step, cnt], [1, n]])

            def xsl_b(col, cnt, step, n):
                # 4B-aligned fp16 slice via parity copies (2x-mode consumers)
                t, c0 = (xe, col) if col % 2 == 0 else (xo, col - 1)
                return ap2d(t[:, c0:c0 + 1], [[step, cnt], [1, n]])

            if DMA_ONLY:
                zstage = sb.tile([128, CH_MAX], F16, tag="zstage")
                nc.vector.memset(zstage[:, :], 0.0)
                for _rep in range(REPS):
                    s = 0
                    for u in _CHUNKS:
                        dst = out_d[s * 128:(s + u) * 128]
                        dst = dst.rearrange("(p f) -> p f", p=128)
                        nc.sync.dma_start(out=dst, in_=zstage[:, 0:u])
                        s += u
            else:
                for _rep in range(REPS):
                    main_pass(nc, stg, tp, ps, xsl_a, xsl_b, lhs, rhs, out_d)

    nc.compile()
    return nc


def main_pass(nc, stg, tp, ps, xsl_a, xsl_b, lhs, rhs, out_d):
    used = 0
    chunk_base = 0
    stage = stg.tile([128, CH_MAX], F16, tag="stage")

    def flush():
        nonlocal used, chunk_base, stage
        if used == 0:
            return
        if not NO_DMA:
            dst = out_d[chunk_base * 128:(chunk_base + used) * 128]
            dst = dst.rearrange("(p f) -> p f", p=128)
            nc.sync.dma_start(out=dst, in_=stage[:, 0:used])
        chunk_base += used
        used = 0
        if chunk_base < P_PAD:
            stage = stg.tile([128, CH_MAX], F16, tag="stage")

    for qi, (j1s, nb, w, js) in enumerate(_QUADS):
        if used + nb * w > CH_MAX:
            flush()
        if DMA_ONLY:
            used += nb * w
            continue
        cls = _quad_class(qi, j1s)
        psum = ps.tile([128, 512 * QUAD], mybir.dt.float32, tag="psum")
        for c in range(nb):
            nc.tensor.matmul(psum[:, 512 * c:512 * c + w],
                             lhs(j1s + c * js), rhs(j1s + c * js, w),
                             start=True, stop=True,
                             tile_position=(32 * ((j1s + c * js) // 128), 0))
        O = used
        if SKIP_CONSUME:
            used += nb * w
            continue
        if cls == "A":
            nc.vector.tensor_mul(
                out=ap2d(stage[:, O:O + 1], [[w, nb], [1, w]]),
                in0=ap2d(psum[:, 0:1], [[512, nb], [1, w]]),
                in1=xsl_a(j1s + 1, nb, js, w))
        else:
            tmp = tp.tile([128, 512 * QUAD], F16, tag="tmp")
            nc.scalar.copy(
                out=ap2d(tmp[:, 0:1], [[512, nb], [1, w]]),
                in_=ap2d(psum[:, 0:1], [[512, nb], [1, w]]))
            if SKIP_MULT:
                used += nb * w
                continue
            if cls == "B" and js == 2:
                # same-parity pair: one 2x-mode op covers both blocks
                nc.vector.tensor_mul(
                    out=ap2d(stage[:, O:O + 1], [[w, nb], [1, w]]),
                    in0=ap2d(tmp[:, 0:1], [[512, nb], [1, w]]),
                    in1=xsl_b(j1s + 1, nb, 2, w))
            elif cls == "B":
                ne = (nb + 1) // 2   # even c: 0, 2
                no = nb // 2         # odd  c: 1, 3
                nc.vector.tensor_mul(
                    out=ap2d(stage[:, O:O + 1], [[2 * w, ne], [1, w]]),
                    in0=ap2d(tmp[:, 0:1], [[1024, ne], [1, w]]),
                    in1=xsl_b(j1s + 1, ne, 2, w))
                if no:
                    nc.vector.tensor_mul(
                        out=ap2d(stage[:, O + w:O + w + 1],
                                 [[2 * w, no], [1, w]]),
                        in0=ap2d(tmp[:, 512:513], [[1024, no], [1, w]]),
                        in1=xsl_b(j1s + 2, no, 2, w))
            else:  # C
                nc.gpsimd.tensor_mul(
                    out=ap2d(stage[:, O:O + 1], [[w, nb], [1, w]]),
                    in0=ap2d(tmp[:, 0:1], [[512, nb], [1, w]]),
                    in1=xsl_a(j1s + 1, nb, js, w))
        used += nb * w
    flush()


_NC_CACHE = None


def kernel(x, weight):
    global _NC_CACHE, LAST_RESULT, _last_in_maps
    x = np.ascontiguousarray(x, dtype=np.float32)
    weight = np.ascontiguousarray(weight, dtype=np.float32)
    assert x.shape == (B, NF) and weight.shape == (NF, K)

    # ---- host-side weight-derived constants (fp16)
    i1, i2 = np.triu_indices(NF, k=1)
    wp_full = weight[i2].T.astype(_f16np)                # [K, P] = w[j2, k]
    wp_in = {}
    for g in range(4):
        arr = np.zeros((K, _GW[g] + _WPAD), dtype=_f16np)
        arr[:, 0:_GW[g]] = wp_full[:, _GOFF[g]:_GOFF[g + 1]]
        wp_in[f"wp{g}"] = arr

    # ---- per-core inputs
    in_maps = []
    for c in range(NCORES):
        xc = x[c * BS:(c + 1) * BS]                      # [128, 512] f32
        xh = xc.astype(_f16np)
        xs2 = np.zeros((2, BS, NF + XPAD), dtype=_f16np)
        xs2[0, :, 0:NF] = xh
        xs2[1, :, 0:NF - 1] = xh[:, 1:]
        xct = xc.T                                        # [512, 128] f32
        xt4 = np.empty((4, K, 128 * 128), dtype=_f16np)
        for g in range(4):
            for k in range(K):
                # lhsT[k, r*128 + b] = x[b, j1] * w[j1, k], j1-major rows
                zk = (xct[128 * g:128 * (g + 1)]
                      * weight[128 * g:128 * (g + 1), k:k + 1]).astype(_f16np)
                xt4[g, k] = zk.reshape(-1)
        m = {"xs2": xs2, "xt4": xt4}
        m.update(wp_in)
        in_maps.append(m)

    _last_in_maps = in_maps
    if _NC_CACHE is None:
        _NC_CACHE = _build_nc()
    nc = _NC_CACHE

    res = bass_utils.run_bass_kernel_spmd(nc, in_maps,
                                          core_ids=list(range(NCORES)),
                                          trace=TRACE)
    LAST_RESULT = {"exec_time_ns": res.exec_time_ns,
                   "trace": res.instructions_and_trace}

    # ---- host unpack: de-chunk, de-pad, upcast
    idx = np.empty(P, dtype=np.int64)
    for j1, (po, n) in enumerate(
            (_POFF[j], NF - 1 - j) for j in range(NF - 1)):
        idx[_off(j1):_off(j1) + n] = np.arange(po, po + n)
    cores = []
    for r in res.results:
        flat = r["out"].reshape(-1)
        b2 = np.empty((BS, P_PAD), dtype=_f16np)
        s = 0
        for u in _CHUNKS:
            b2[:, s:s + u] = flat[128 * s:128 * (s + u)].reshape(BS, u)
            s += u
        cores.append(b2)
    blob_all = np.concatenate(cores, axis=0)              # [1024, P_PAD]
    out = blob_all[:, idx].astype(np.float32)
    return out


# revision 26
# speedup vs baseline: 1.8482x; 1.3738x over previous
"""FM pairwise-interaction layer on 8 Trainium2 NeuronCores (fp16 pipeline).

out[b, p] = x[b, I1[p]] * x[b, I2[p]] * dot(w[I1[p]], w[I2[p]])   for all
P = 512*511/2 = 130816 strict upper-triangle pairs, batch 1024.

Strategy (data-parallel over batch, 128 rows per core):
  *  wdot = W @ W.T has rank 4, so a K=4 fp16 matmul per j1-block computes
        psum[b, c] = sum_k (x[b,j1] w[j1,k]) * (w[j2,k])  =  x[b,j1]*wdot[p]
     with x.T and WP[k,p] = w[I2,k] shipped fp16 from the host.
  *  Output is fp16 (rel err ~7e-4 << 2e-2 gate); host upcasts.  This
     halves the HBM write roofline vs f32 (33.6 MB/core, ~103 us at the
     measured ~330 GB/s per-core DMA write bandwidth).
  *  The PE on this part runs at 1.2 GHz (no HAM ramp observed), so serial
     matmuls are ~112 us.  Fix: blocks are processed round-robin across the
     four 128-row j1-groups; their lhsT/rhs data live at partition offsets
     32g, so consecutive matmuls hit different PE row-groups
     (tile_position) and execute concurrently (measured 82 us, 4 PSUM
     tiles in flight).
  *  Each unit is a PAIR of same-parity blocks (j1, j1+2) -> one 2-bank
     PSUM tile, slot width w = ceil_even(n0); merged 2D-AP consumer ops
     amortize per-op init overhead, and the parity stride keeps fp16
     operands 4B-aligned for DVE 2x mode.
  *  Per-unit consumer classes (tunable split via J_SPLIT):
       A: DVE tensor_mul direct from PSUM (1x mode, no ACT involvement)
       B: ACT evacuates PSUM->SBUF fp16, then one DVE tensor_mul at 2x
          mode (all operands fp16, inner step 1, 4B-aligned)
     balancing DVE (~1.04 cyc/elem A-path) against ACT+DVE (~1.0+0.5).
     GPSIMD is not used: its SBUF port is locked out by every DVE
     tensor_tensor op (measured -20 us when enabled).
"""

import numpy as np

import concourse.bass as bass
import concourse.mybir as mybir
from concourse import bacc
from concourse.tile import TileContext
import concourse.bass_utils as bass_utils

NF = 512          # features
K = 4             # latent dim
B = 1024          # batch
NCORES = 8
BS = B // NCORES  # 128 batch rows per core
P = NF * (NF - 1) // 2  # 130816 pairs

F16 = mybir.dt.float16
_f16np = np.float16

QUAD = 2          # blocks per PSUM tile unit (parity-strided)
J_SPLIT = 260     # j1 < J_SPLIT -> class B/C (ACT-assisted), else class A
C_EVERY = 2      # every C_EVERY-th A-region quad goes to GPSIMD (0 = off)
CH_MAX = 6144     # stage tile columns (fp16); flush threshold
XPAD = 8
STAGE_BUFS = 3
TMP_BUFS = 8
PSUM_BUFS = 4     # [128, 512*QUAD] tiles; QUAD*PSUM_BUFS <= 8 banks
REPS = 1
DMA_ONLY = 0      # 1: skip all compute, only DMA flushes (timing experiments)
NO_DMA = 0        # 1: skip output flush DMAs (timing experiments)
SKIP_CONSUME = 0  # 1: matmuls only, no consumer ops (timing experiments)
SKIP_MULT = 0     # 1: matmuls + ACT evacs only, no DVE/GPS mults (timing)
TRACE = False
LAST_RESULT = {}
_last_in_maps = None

_WPAD = 8


def _off(j1):
    return j1 * (NF - 1) - j1 * (j1 - 1) // 2


_GOFF = [_off(0), _off(128), _off(256), _off(384), P]
_GW = [_GOFF[g + 1] - _GOFF[g] for g in range(4)]  # 57280, 40896, 24512, 8128


def _quads():
    """Processing order: round-robin across the 4 j1-groups so consecutive
    matmuls land on different PE row-groups (tile_position concurrency).
    Each unit is QUAD same-parity blocks (j1, j1+2, ..) from one group, so
    the class-B multiply is a single 2x-mode DVE op (xs cols j1+1, j1+3, ..
    share parity -> 4B-aligned outer step 2).  Units come in low/high parity
    couples covering 2*QUAD consecutive blocks."""
    def ew(j1):
        n = NF - 1 - j1
        return n + (n & 1)

    cursors = [128 * g for g in range(4)]
    limits = [min(128 * (g + 1), NF - 1) for g in range(4)]
    pend = [None] * 4   # deferred odd-parity unit per group
    out = []
    while True:
        alive = False
        for g in range(4):
            if pend[g] is not None:
                out.append(pend[g])
                pend[g] = None
                alive = True
                continue
            j1 = cursors[g]
            if j1 >= limits[g]:
                continue
            alive = True
            rem = limits[g] - j1
            if rem >= 2 * QUAD:
                out.append((j1, QUAD, ew(j1), 2))
                pend[g] = (j1 + 1, QUAD, ew(j1 + 1), 2)
                cursors[g] += 2 * QUAD
            elif rem >= 2:
                ne = (rem + 1) // 2
                no = rem // 2
                out.append((j1, ne, ew(j1), 2))
                if no:
                    pend[g] = (j1 + 1, no, ew(j1 + 1), 2)
                cursors[g] += rem
            else:
                out.append((j1, 1, ew(j1), 1))
                cursors[g] += 1
        if not alive:
            break
    return out


def _relayout():
    """Recompute processing order / stage layout from current knobs."""
    global _QUADS, _POFF, P_PAD, _CHUNKS
    _QUADS = _quads()
    _POFF = {}
    pp = 0
    for j1s, nb, w, js in _QUADS:
        for c in range(nb):
            _POFF[j1s + c * js] = pp
            pp += w
    P_PAD = pp
    _CHUNKS = []
    acc = 0
    for j1s, nb, w, js in _QUADS:
        if acc + nb * w > CH_MAX:
            _CHUNKS.append(acc)
            acc = 0
        acc += nb * w
    if acc:
        _CHUNKS.append(acc)
    assert sum(_CHUNKS) == P_PAD


_relayout()


def ap2d(sliced, dims):
    """Copy of AP `sliced` with its free dims replaced by [step, count] pairs."""
    c = sliced.copy()
    v = c.ap
    part = [list(v[0])]
    while len(v) > 0:
        v.pop()
    for d in part + [list(x) for x in dims]:
        v.append(d)
    c.ap = v
    return c


A_EVERY = 0       # among B-region quads, every A_EVERY-th becomes class A


def _quad_class(qi, j1):
    if j1 >= J_SPLIT:
        # C-units (ACT evac + GpSimd mult) live in the A-region, where DVE
        # ops are PSUM-sourced and leave the shared SBUF port free
        if C_EVERY and qi % C_EVERY == C_EVERY - 1:
            return "C"
        return "A"
    if A_EVERY and qi % A_EVERY == A_EVERY - 1:
        return "A"
    return "B"


def _build_nc():
    nc = bacc.Bacc("TRN2", target_bir_lowering=False, debug=False,
                   num_devices=NCORES)

    xs_d = nc.dram_tensor("xs2", (2, BS, NF + XPAD), F16,
                          kind="ExternalInput").ap()
    xt_d = nc.dram_tensor("xt4", (4, K, 128 * 128), F16,
                          kind="ExternalInput").ap()
    wp_d = [nc.dram_tensor(f"wp{g}", (K, _GW[g] + _WPAD), F16,
                           kind="ExternalInput").ap() for g in range(4)]
    out_d = nc.dram_tensor("out", (BS * P_PAD,), F16, kind="ExternalOutput").ap()

    with TileContext(nc) as tc:
        with tc.tile_pool(name="sb", bufs=1) as sb, \
             tc.tile_pool(name="stg", bufs=STAGE_BUFS) as stg, \
             tc.tile_pool(name="tp", bufs=TMP_BUFS) as tp, \
             tc.tile_pool(name="ps", bufs=PSUM_BUFS, space="PSUM") as ps:

            xe = sb.tile([128, NF + XPAD], F16, tag="xe")
            xo = sb.tile([128, NF + XPAD], F16, tag="xo")
            nc.sync.dma_start(out=xe[:, :], in_=xs_d[0])
            nc.sync.dma_start(out=xo[:, :], in_=xs_d[1])

            xt = sb.tile([128, 128 * 128], F16, tag="xt")
            wp = sb.tile([128, _GW[0] + _WPAD], F16, tag="wp")
            for g in range(4):
                nc.sync.dma_start(out=xt[32 * g:32 * g + K, :], in_=xt_d[g])
                nc.sync.dma_start(out=wp[32 * g:32 * g + K, 0:_GW[g] + _WPAD],
                                  in_=wp_d[g][:])

            def lhs(j1):
                g = j1 // 128
                r = j1 - 128 * g
                return xt[32 * g:32 * g + K, r * 128:(r + 1) * 128]

            def rhs(j1, n):
                g = j1 // 128
                lo = _off(j1) - _GOFF[g]
                return wp[32 * g:32 * g + K, lo:lo + n]

            def xsl_a(col, cnt, step, n):
                # alignment-free slice (1x-mode consumers)
                return ap2d(xe[:, col:col + 1], [[step, cnt], [1, n]])

            def xsl_b(col, cnt, step, n):
                # 4B-aligned fp16 slice via parity copies (2x-mode consumers)
                t, c0 = (xe, col) if col % 2 == 0 else (xo, col - 1)
                return ap2d(t[:, c0:c0 + 1], [[step, cnt], [1, n]])

            if DMA_ONLY:
                zstage = sb.tile([128, CH_MAX], F16, tag="zstage")
                nc.vector.memset(zstage[:, :], 0.0)
                for _rep in range(REPS):
                    s = 0
                    for u in _CHUNKS:
                        dst = out_d[s * 128:(s + u) * 128]
                        dst = dst.rearrange("(p f) -> p f", p=128)
                        nc.sync.dma_start(out=dst, in_=zstage[:, 0:u])
                        s += u
            else:
                for _rep in range(REPS):
                    main_pass(nc, stg, tp, ps, xsl_a, xsl_b, lhs, rhs, out_d)

    nc.compile()
    return nc


def main_pass(nc, stg, tp, ps, xsl_a, xsl_b, lhs, rhs, out_d):
    used = 0
    chunk_base = 0
    stage = stg.tile([128, CH_MAX], F16, tag="stage")

    def flush():
        nonlocal used, chunk_base, stage
        if used == 0:
            return
        if not NO_DMA:
            dst = out_d[chunk_base * 128:(chunk_base + used) * 128]
            dst = dst.rearrange("(p f) -> p f", p=128)
            nc.sync.dma_start(out=dst, in_=stage[:, 0:used])
        chunk_base += used
        used = 0
        if chunk_base < P_PAD:
            stage = stg.tile([128, CH_MAX], F16, tag="stage")

    for qi, (j1s, nb, w, js) in enumerate(_QUADS):
        if used + nb * w > CH_MAX:
            flush()
        if DMA_ONLY:
            used += nb * w
            continue
        cls = _quad_class(qi, j1s)
        psum = ps.tile([128, 512 * QUAD], mybir.dt.float32, tag="psum")
        for c in range(nb):
            nc.tensor.matmul(psum[:, 512 * c:512 * c + w],
                             lhs(j1s + c * js), rhs(j1s + c * js, w),
                             start=True, stop=True,
                             tile_position=(32 * ((j1s + c * js) // 128), 0))
        O = used
        if SKIP_CONSUME:
            used += nb * w
            continue
        if cls == "A":
            nc.vector.tensor_mul(
                out=ap2d(stage[:, O:O + 1], [[w, nb], [1, w]]),
                in0=ap2d(psum[:, 0:1], [[512, nb], [1, w]]),
                in1=xsl_a(j1s + 1, nb, js, w))
        else:
            tmp = tp.tile([128, 512 * QUAD], F16, tag="tmp")
            nc.scalar.copy(
                out=ap2d(tmp[:, 0:1], [[512, nb], [1, w]]),
                in_=ap2d(psum[:, 0:1], [[512, nb], [1, w]]))
            if SKIP_MULT:
                used += nb * w
                continue
            if cls == "B" and js == 2:
                # same-parity pair: one 2x-mode op covers both blocks
                nc.vector.tensor_mul(
                    out=ap2d(stage[:, O:O + 1], [[w, nb], [1, w]]),
                    in0=ap2d(tmp[:, 0:1], [[512, nb], [1, w]]),
                    in1=xsl_b(j1s + 1, nb, 2, w))
            elif cls == "B":
                ne = (nb + 1) // 2   # even c: 0, 2
                no = nb // 2         # odd  c: 1, 3
                nc.vector.tensor_mul(
                    out=ap2d(stage[:, O:O + 1], [[2 * w, ne], [1, w]]),
                    in0=ap2d(tmp[:, 0:1], [[1024, ne], [1, w]]),
                    in1=xsl_b(j1s + 1, ne, 2, w))
                if no:
                    nc.vector.tensor_mul(
                        out=ap2d(stage[:, O + w:O + w + 1],
                                 [[2 * w, no], [1, w]]),
                        in0=ap2d(tmp[:, 512:513], [[1024, no], [1, w]]),
                        in1=xsl_b(j1s + 2, no, 2, w))
            else:  # C
                nc.gpsimd.tensor_mul(
                    out=ap2d(stage[:, O:O + 1], [[w, nb], [1, w]]),
                    in0=ap2d(tmp[:, 0:1], [[512, nb], [1, w]]),
                    in1=xsl_a(j1s + 1, nb, js, w))
        used += nb * w
    flush()


_NC_CACHE = None


def kernel(x, weight):
    global _NC_CACHE, LAST_RESULT, _last_in_maps
    x = np.ascontiguousarray(x, dtype=np.float32)
    weight = np.ascontiguousarray(weight, dtype=np.float32)
    assert x.shape == (B, NF) and weight.shape == (NF, K)

    # ---- host-side weight-derived constants (fp16)
    i1, i2 = np.triu_indices(NF, k=1)
    wp_full = weight[i2].T.astype(_f16np)                # [K, P] = w[j2, k]
    wp_in = {}
    for g in range(4):
        arr = np.zeros((K, _GW[g] + _WPAD), dtype=_f16np)
        arr[:, 0:_GW[g]] = wp_full[:, _GOFF[g]:_GOFF[g + 1]]
        wp_in[f"wp{g}"] = arr

    # ---- per-core inputs
    in_maps = []
    for c in range(NCORES):
        xc = x[c * BS:(c + 1) * BS]                      # [128, 512] f32
        xh = xc.astype(_f16np)
        xs2 = np.zeros((2, BS, NF + XPAD), dtype=_f16np)
        xs2[0, :, 0:NF] = xh
        xs2[1, :, 0:NF - 1] = xh[:, 1:]
        xct = xc.T                                        # [512, 128] f32
        xt4 = np.empty((4, K, 128 * 128), dtype=_f16np)
        for g in range(4):
            for k in range(K):
                # lhsT[k, r*128 + b] = x[b, j1] * w[j1, k], j1-major rows
                zk = (xct[128 * g:128 * (g + 1)]
                      * weight[128 * g:128 * (g + 1), k:k + 1]).astype(_f16np)
                xt4[g, k] = zk.reshape(-1)
        m = {"xs2": xs2, "xt4": xt4}
        m.update(wp_in)
        in_maps.append(m)

    _last_in_maps = in_maps
    if _NC_CACHE is None:
        _NC_CACHE = _build_nc()
    nc = _NC_CACHE

    res = bass_utils.run_bass_kernel_spmd(nc, in_maps,
                                          core_ids=list(range(NCORES)),
                                          trace=TRACE)
    LAST_RESULT = {"exec_time_ns": res.exec_time_ns,
                   "trace": res.instructions_and_trace}

    # ---- host unpack: de-chunk, de-pad, upcast
    idx = np.empty(P, dtype=np.int64)
    for j1, (po, n) in enumerate(
            (_POFF[j], NF - 1 - j) for j in range(NF - 1)):
        idx[_off(j1):_off(j1) + n] = np.arange(po, po + n)
    cores = []
    for r in res.results:
        flat = r["out"].reshape(-1)
        b2 = np.empty((BS, P_PAD), dtype=_f16np)
        s = 0
        for u in _CHUNKS:
            b2[:, s:s + u] = flat[128 * s:128 * (s + u)].reshape(BS, u)
            s += u
        cores.append(b2)
    blob_all = np.concatenate(cores, axis=0)              # [1024, P_PAD]
    out = blob_all[:, idx].astype(np.float32)
    return out


# revision 27
# speedup vs baseline: 2.0706x; 1.1204x over previous
"""FM pairwise-interaction layer on 8 Trainium2 NeuronCores (fp16 pipeline).

out[b, p] = x[b, I1[p]] * x[b, I2[p]] * dot(w[I1[p]], w[I2[p]])   for all
P = 512*511/2 = 130816 strict upper-triangle pairs, batch 1024.

Strategy (data-parallel over batch, 128 rows per core):
  *  wdot = W @ W.T has rank 4, so a K=4 fp16 matmul per j1-block computes
        psum[b, c] = sum_k (x[b,j1] w[j1,k]) * (w[j2,k])  =  x[b,j1]*wdot[p]
     with x.T and WP[k,p] = w[I2,k] shipped fp16 from the host.
  *  Output is fp16 (rel err ~7e-4 << 2e-2 gate); host upcasts.  This
     halves the HBM write roofline vs f32 (33.6 MB/core, ~103 us at the
     measured ~330 GB/s per-core DMA write bandwidth).
  *  The PE on this part runs at 1.2 GHz (no HAM ramp observed), so serial
     matmuls are ~112 us.  Fix: blocks are processed round-robin across the
     four 128-row j1-groups; their lhsT/rhs data live at partition offsets
     32g, so consecutive matmuls hit different PE row-groups
     (tile_position) and execute concurrently (measured 82 us, 4 PSUM
     tiles in flight).
  *  Each unit is a PAIR of same-parity blocks (j1, j1+2) -> one 2-bank
     PSUM tile, slot width w = ceil_even(n0); merged 2D-AP consumer ops
     amortize per-op init overhead, and the parity stride keeps fp16
     operands 4B-aligned for DVE 2x mode.
  *  Per-unit consumer classes (tunable split via J_SPLIT):
       A: DVE tensor_mul direct from PSUM (1x mode, no ACT involvement)
       B: ACT evacuates PSUM->SBUF fp16, then one DVE tensor_mul at 2x
          mode (all operands fp16, inner step 1, 4B-aligned)
     balancing DVE (~1.04 cyc/elem A-path) against ACT+DVE (~1.0+0.5).
     GPSIMD is not used: its SBUF port is locked out by every DVE
     tensor_tensor op (measured -20 us when enabled).
"""

import numpy as np

import concourse.bass as bass
import concourse.mybir as mybir
from concourse import bacc
from concourse.tile import TileContext
import concourse.bass_utils as bass_utils

NF = 512          # features
K = 4             # latent dim
B = 1024          # batch
NCORES = 8
BS = B // NCORES  # 128 batch rows per core
P = NF * (NF - 1) // 2  # 130816 pairs

F16 = mybir.dt.float16
_f16np = np.float16

QUAD = 2          # blocks per PSUM tile unit (parity-strided)
J_SPLIT = 270     # j1 < J_SPLIT -> class B (ACT-assisted), else class A/C
C_EVERY = 3      # every C_EVERY-th A-region quad goes to GPSIMD (0 = off)
CH_MAX = 6144     # stage tile columns (fp16); flush threshold
XPAD = 8
STAGE_BUFS = 3
TMP_BUFS = 8
PSUM_BUFS = 4     # [128, 512*QUAD] tiles; QUAD*PSUM_BUFS <= 8 banks
REPS = 1
DMA_ONLY = 0      # 1: skip all compute, only DMA flushes (timing experiments)
NO_DMA = 0        # 1: skip output flush DMAs (timing experiments)
SKIP_CONSUME = 0  # 1: matmuls only, no consumer ops (timing experiments)
SKIP_MULT = 0     # 1: matmuls + ACT evacs only, no DVE/GPS mults (timing)
TRACE = False
LAST_RESULT = {}
_last_in_maps = None

_WPAD = 8


def _off(j1):
    return j1 * (NF - 1) - j1 * (j1 - 1) // 2


_GOFF = [_off(0), _off(128), _off(256), _off(384), P]
_GW = [_GOFF[g + 1] - _GOFF[g] for g in range(4)]  # 57280, 40896, 24512, 8128


def _quads():
    """Processing order: round-robin across the 4 j1-groups so consecutive
    matmuls land on different PE row-groups (tile_position concurrency).
    Each unit is QUAD same-parity blocks (j1, j1+2, ..) from one group, so
    the class-B multiply is a single 2x-mode DVE op (xs cols j1+1, j1+3, ..
    share parity -> 4B-aligned outer step 2).  Units come in low/high parity
    couples covering 2*QUAD consecutive blocks."""
    def ew(j1):
        n = NF - 1 - j1
        return n + (n & 1)

    cursors = [128 * g for g in range(4)]
    limits = [min(128 * (g + 1), NF - 1) for g in range(4)]
    pend = [None] * 4   # deferred odd-parity unit per group
    out = []
    while True:
        alive = False
        for g in range(4):
            if pend[g] is not None:
                out.append(pend[g])
                pend[g] = None
                alive = True
                continue
            j1 = cursors[g]
            if j1 >= limits[g]:
                continue
            alive = True
            rem = limits[g] - j1
            if rem >= 2 * QUAD:
                out.append((j1, QUAD, ew(j1), 2))
                pend[g] = (j1 + 1, QUAD, ew(j1 + 1), 2)
                cursors[g] += 2 * QUAD
            elif rem >= 2:
                ne = (rem + 1) // 2
                no = rem // 2
                out.append((j1, ne, ew(j1), 2))
                if no:
                    pend[g] = (j1 + 1, no, ew(j1 + 1), 2)
                cursors[g] += rem
            else:
                out.append((j1, 1, ew(j1), 1))
                cursors[g] += 1
        if not alive:
            break
    return out


def _relayout():
    """Recompute processing order / stage layout from current knobs."""
    global _QUADS, _POFF, P_PAD, _CHUNKS
    _QUADS = _quads()
    _POFF = {}
    pp = 0
    for j1s, nb, w, js in _QUADS:
        for c in range(nb):
            _POFF[j1s + c * js] = pp
            pp += w
    P_PAD = pp
    _CHUNKS = []
    acc = 0
    for j1s, nb, w, js in _QUADS:
        if acc + nb * w > CH_MAX:
            _CHUNKS.append(acc)
            acc = 0
        acc += nb * w
    if acc:
        _CHUNKS.append(acc)
    assert sum(_CHUNKS) == P_PAD


_relayout()


def ap2d(sliced, dims):
    """Copy of AP `sliced` with its free dims replaced by [step, count] pairs."""
    c = sliced.copy()
    v = c.ap
    part = [list(v[0])]
    while len(v) > 0:
        v.pop()
    for d in part + [list(x) for x in dims]:
        v.append(d)
    c.ap = v
    return c


A_EVERY = 0       # among B-region quads, every A_EVERY-th becomes class A


def _quad_class(qi, j1):
    if j1 >= J_SPLIT:
        # C-units (ACT evac + GpSimd mult) live in the A-region, where DVE
        # ops are PSUM-sourced and leave the shared SBUF port free
        if C_EVERY and qi % C_EVERY == C_EVERY - 1:
            return "C"
        return "A"
    if A_EVERY and qi % A_EVERY == A_EVERY - 1:
        return "A"
    return "B"


def _build_nc():
    nc = bacc.Bacc("TRN2", target_bir_lowering=False, debug=False,
                   num_devices=NCORES)

    xs_d = nc.dram_tensor("xs2", (2, BS, NF + XPAD), F16,
                          kind="ExternalInput").ap()
    xt_d = nc.dram_tensor("xt4", (4, K, 128 * 128), F16,
                          kind="ExternalInput").ap()
    wp_d = [nc.dram_tensor(f"wp{g}", (K, _GW[g] + _WPAD), F16,
                           kind="ExternalInput").ap() for g in range(4)]
    out_d = nc.dram_tensor("out", (BS * P_PAD,), F16, kind="ExternalOutput").ap()

    with TileContext(nc) as tc:
        with tc.tile_pool(name="sb", bufs=1) as sb, \
             tc.tile_pool(name="stg", bufs=STAGE_BUFS) as stg, \
             tc.tile_pool(name="tp", bufs=TMP_BUFS) as tp, \
             tc.tile_pool(name="ps", bufs=PSUM_BUFS, space="PSUM") as ps:

            xe = sb.tile([128, NF + XPAD], F16, tag="xe")
            xo = sb.tile([128, NF + XPAD], F16, tag="xo")
            nc.sync.dma_start(out=xe[:, :], in_=xs_d[0])
            nc.sync.dma_start(out=xo[:, :], in_=xs_d[1])

            xt = sb.tile([128, 128 * 128], F16, tag="xt")
            wp = sb.tile([128, _GW[0] + _WPAD], F16, tag="wp")
            for g in range(4):
                nc.sync.dma_start(out=xt[32 * g:32 * g + K, :], in_=xt_d[g])
                nc.sync.dma_start(out=wp[32 * g:32 * g + K, 0:_GW[g] + _WPAD],
                                  in_=wp_d[g][:])

            def lhs(j1):
                g = j1 // 128
                r = j1 - 128 * g
                return xt[32 * g:32 * g + K, r * 128:(r + 1) * 128]

            def rhs(j1, n):
                g = j1 // 128
                lo = _off(j1) - _GOFF[g]
                return wp[32 * g:32 * g + K, lo:lo + n]

            def xsl_a(col, cnt, step, n):
                # alignment-free slice (1x-mode consumers)
                return ap2d(xe[:, col:col + 1], [[step, cnt], [1, n]])

            def xsl_b(col, cnt, step, n):
                # 4B-aligned fp16 slice via parity copies (2x-mode consumers)
                t, c0 = (xe, col) if col % 2 == 0 else (xo, col - 1)
                return ap2d(t[:, c0:c0 + 1], [[step, cnt], [1, n]])

            if DMA_ONLY:
                zstage = sb.tile([128, CH_MAX], F16, tag="zstage")
                nc.vector.memset(zstage[:, :], 0.0)
                for _rep in range(REPS):
                    s = 0
                    for u in _CHUNKS:
                        dst = out_d[s * 128:(s + u) * 128]
                        dst = dst.rearrange("(p f) -> p f", p=128)
                        nc.sync.dma_start(out=dst, in_=zstage[:, 0:u])
                        s += u
            else:
                for _rep in range(REPS):
                    main_pass(nc, stg, tp, ps, xsl_a, xsl_b, lhs, rhs, out_d)

    nc.compile()
    return nc


def main_pass(nc, stg, tp, ps, xsl_a, xsl_b, lhs, rhs, out_d):
    used = 0
    chunk_base = 0
    stage = stg.tile([128, CH_MAX], F16, tag="stage")

    def flush():
        nonlocal used, chunk_base, stage
        if used == 0:
            return
        if not NO_DMA:
            dst = out_d[chunk_base * 128:(chunk_base + used) * 128]
            dst = dst.rearrange("(p f) -> p f", p=128)
            nc.sync.dma_start(out=dst, in_=stage[:, 0:used])
        chunk_base += used
        used = 0
        if chunk_base < P_PAD:
            stage = stg.tile([128, CH_MAX], F16, tag="stage")

    for qi, (j1s, nb, w, js) in enumerate(_QUADS):
        if used + nb * w > CH_MAX:
            flush()
        if DMA_ONLY:
            used += nb * w
            continue
        cls = _quad_class(qi, j1s)
        psum = ps.tile([128, 512 * QUAD], mybir.dt.float32, tag="psum")
        for c in range(nb):
            nc.tensor.matmul(psum[:, 512 * c:512 * c + w],
                             lhs(j1s + c * js), rhs(j1s + c * js, w),
                             start=True, stop=True,
                             tile_position=(32 * ((j1s + c * js) // 128), 0))
        O = used
        if SKIP_CONSUME:
            used += nb * w
            continue
        if cls == "A":
            nc.vector.tensor_mul(
                out=ap2d(stage[:, O:O + 1], [[w, nb], [1, w]]),
                in0=ap2d(psum[:, 0:1], [[512, nb], [1, w]]),
                in1=xsl_a(j1s + 1, nb, js, w))
        else:
            tmp = tp.tile([128, 512 * QUAD], F16, tag="tmp")
            nc.scalar.copy(
                out=ap2d(tmp[:, 0:1], [[512, nb], [1, w]]),
                in_=ap2d(psum[:, 0:1], [[512, nb], [1, w]]))
            if SKIP_MULT:
                used += nb * w
                continue
            if cls == "B" and js == 2:
                # same-parity pair: one 2x-mode op covers both blocks
                nc.vector.tensor_mul(
                    out=ap2d(stage[:, O:O + 1], [[w, nb], [1, w]]),
                    in0=ap2d(tmp[:, 0:1], [[512, nb], [1, w]]),
                    in1=xsl_b(j1s + 1, nb, 2, w))
            elif cls == "B":
                ne = (nb + 1) // 2   # even c: 0, 2
                no = nb // 2         # odd  c: 1, 3
                nc.vector.tensor_mul(
                    out=ap2d(stage[:, O:O + 1], [[2 * w, ne], [1, w]]),
                    in0=ap2d(tmp[:, 0:1], [[1024, ne], [1, w]]),
                    in1=xsl_b(j1s + 1, ne, 2, w))
                if no:
                    nc.vector.tensor_mul(
                        out=ap2d(stage[:, O + w:O + w + 1],
                                 [[2 * w, no], [1, w]]),
                        in0=ap2d(tmp[:, 512:513], [[1024, no], [1, w]]),
                        in1=xsl_b(j1s + 2, no, 2, w))
            else:  # C
                nc.gpsimd.tensor_mul(
                    out=ap2d(stage[:, O:O + 1], [[w, nb], [1, w]]),
                    in0=ap2d(tmp[:, 0:1], [[512, nb], [1, w]]),
                    in1=xsl_a(j1s + 1, nb, js, w))
        used += nb * w
    flush()


_NC_CACHE = None


def kernel(x, weight):
    global _NC_CACHE, LAST_RESULT, _last_in_maps
    x = np.ascontiguousarray(x, dtype=np.float32)
    weight = np.ascontiguousarray(weight, dtype=np.float32)
    assert x.shape == (B, NF) and weight.shape == (NF, K)

    # ---- host-side weight-derived constants (fp16)
    i1, i2 = np.triu_indices(NF, k=1)
    wp_full = weight[i2].T.astype(_f16np)                # [K, P] = w[j2, k]
    wp_in = {}
    for g in range(4):
        arr = np.zeros((K, _GW[g] + _WPAD), dtype=_f16np)
        arr[:, 0:_GW[g]] = wp_full[:, _GOFF[g]:_GOFF[g + 1]]
        wp_in[f"wp{g}"] = arr

    # ---- per-core inputs
    in_maps = []
    for c in range(NCORES):
        xc = x[c * BS:(c + 1) * BS]                      # [128, 512] f32
        xh = xc.astype(_f16np)
        xs2 = np.zeros((2, BS, NF + XPAD), dtype=_f16np)
        xs2[0, :, 0:NF] = xh
        xs2[1, :, 0:NF - 1] = xh[:, 1:]
        xct = xc.T                                        # [512, 128] f32
        xt4 = np.empty((4, K, 128 * 128), dtype=_f16np)
        for g in range(4):
            for k in range(K):
                # lhsT[k, r*128 + b] = x[b, j1] * w[j1, k], j1-major rows
                zk = (xct[128 * g:128 * (g + 1)]
                      * weight[128 * g:128 * (g + 1), k:k + 1]).astype(_f16np)
                xt4[g, k] = zk.reshape(-1)
        m = {"xs2": xs2, "xt4": xt4}
        m.update(wp_in)
        in_maps.append(m)

    _last_in_maps = in_maps
    if _NC_CACHE is None:
        _NC_CACHE = _build_nc()
    nc = _NC_CACHE

    res = bass_utils.run_bass_kernel_spmd(nc, in_maps,
                                          core_ids=list(range(NCORES)),
                                          trace=TRACE)
    LAST_RESULT = {"exec_time_ns": res.exec_time_ns,
                   "trace": res.instructions_and_trace}

    # ---- host unpack: de-chunk, de-pad, upcast
    idx = np.empty(P, dtype=np.int64)
    for j1, (po, n) in enumerate(
            (_POFF[j], NF - 1 - j) for j in range(NF - 1)):
        idx[_off(j1):_off(j1) + n] = np.arange(po, po + n)
    cores = []
    for r in res.results:
        flat = r["out"].reshape(-1)
        b2 = np.empty((BS, P_PAD), dtype=_f16np)
        s = 0
        for u in _CHUNKS:
            b2[:, s:s + u] = flat[128 * s:128 * (s + u)].reshape(BS, u)
            s += u
        cores.append(b2)
    blob_all = np.concatenate(cores, axis=0)              # [1024, P_PAD]
    out = blob_all[:, idx].astype(np.float32)
    return out


# revision 28
# speedup vs baseline: 2.0799x; 1.0045x over previous
"""FM pairwise-interaction layer on 8 Trainium2 NeuronCores (fp16 pipeline).

out[b, p] = x[b, I1[p]] * x[b, I2[p]] * dot(w[I1[p]], w[I2[p]])   for all
P = 512*511/2 = 130816 strict upper-triangle pairs, batch 1024.

Strategy (data-parallel over batch, 128 rows per core):
  *  wdot = W @ W.T has rank 4, so a K=4 fp16 matmul per j1-block computes
        psum[b, c] = sum_k (x[b,j1] w[j1,k]) * (w[j2,k])  =  x[b,j1]*wdot[p]
     with x.T and WP[k,p] = w[I2,k] shipped fp16 from the host.
  *  Output is fp16 (rel err ~7e-4 << 2e-2 gate); host upcasts.  This
     halves the HBM write roofline vs f32 (33.6 MB/core, ~103 us at the
     measured ~330 GB/s per-core DMA write bandwidth).
  *  The PE on this part runs at 1.2 GHz (no HAM ramp observed), so serial
     matmuls are ~112 us.  Fix: blocks are processed round-robin across the
     four 128-row j1-groups; their lhsT/rhs data live at partition offsets
     32g, so consecutive matmuls hit different PE row-groups
     (tile_position) and execute concurrently (measured 82 us, 4 PSUM
     tiles in flight).
  *  Each unit is a PAIR of same-parity blocks (j1, j1+2) -> one 2-bank
     PSUM tile, slot width w = ceil_even(n0); merged 2D-AP consumer ops
     amortize per-op init overhead, and the parity stride keeps fp16
     operands 4B-aligned for DVE 2x mode.
  *  Per-unit consumer classes (tunable split via J_SPLIT):
       A: DVE tensor_mul direct from PSUM (1x mode, no ACT involvement)
       B: ACT evacuates PSUM->SBUF fp16, then one DVE tensor_mul at 2x
          mode (all operands fp16, inner step 1, 4B-aligned)
     balancing DVE (~1.04 cyc/elem A-path) against ACT+DVE (~1.0+0.5).
     GPSIMD is not used: its SBUF port is locked out by every DVE
     tensor_tensor op (measured -20 us when enabled).
"""

import numpy as np

import concourse.bass as bass
import concourse.mybir as mybir
from concourse import bacc
from concourse.tile import TileContext
import concourse.bass_utils as bass_utils

NF = 512          # features
K = 4             # latent dim
B = 1024          # batch
NCORES = 8
BS = B // NCORES  # 128 batch rows per core
P = NF * (NF - 1) // 2  # 130816 pairs

F16 = mybir.dt.float16
_f16np = np.float16

QUAD = 2          # blocks per PSUM tile unit (parity-strided)
J_SPLIT = 270     # j1 < J_SPLIT -> class B (ACT-assisted), else class A/C
C_EVERY = 3      # every C_EVERY-th A-region quad goes to GPSIMD (0 = off)
CH_MAX = 7168     # stage tile columns (fp16); flush threshold
XPAD = 8
STAGE_BUFS = 3
TMP_BUFS = 8
PSUM_BUFS = 4     # [128, 512*QUAD] tiles; QUAD*PSUM_BUFS <= 8 banks
REPS = 1
DMA_ONLY = 0      # 1: skip all compute, only DMA flushes (timing experiments)
NO_DMA = 0        # 1: skip output flush DMAs (timing experiments)
SKIP_CONSUME = 0  # 1: matmuls only, no consumer ops (timing experiments)
SKIP_MULT = 0     # 1: matmuls + ACT evacs only, no DVE/GPS mults (timing)
TRACE = False
LAST_RESULT = {}
_last_in_maps = None

_WPAD = 8


def _off(j1):
    return j1 * (NF - 1) - j1 * (j1 - 1) // 2


_GOFF = [_off(0), _off(128), _off(256), _off(384), P]
_GW = [_GOFF[g + 1] - _GOFF[g] for g in range(4)]  # 57280, 40896, 24512, 8128


def _quads():
    """Processing order: round-robin across the 4 j1-groups so consecutive
    matmuls land on different PE row-groups (tile_position concurrency).
    Each unit is QUAD same-parity blocks (j1, j1+2, ..) from one group, so
    the class-B multiply is a single 2x-mode DVE op (xs cols j1+1, j1+3, ..
    share parity -> 4B-aligned outer step 2).  Units come in low/high parity
    couples covering 2*QUAD consecutive blocks."""
    def ew(j1):
        n = NF - 1 - j1
        return n + (n & 1)

    cursors = [128 * g for g in range(4)]
    limits = [min(128 * (g + 1), NF - 1) for g in range(4)]
    pend = [None] * 4   # deferred odd-parity unit per group
    out = []
    while True:
        alive = False
        for g in range(4):
            if pend[g] is not None:
                out.append(pend[g])
                pend[g] = None
                alive = True
                continue
            j1 = cursors[g]
            if j1 >= limits[g]:
                continue
            alive = True
            rem = limits[g] - j1
            if rem >= 2 * QUAD:
                out.append((j1, QUAD, ew(j1), 2))
                pend[g] = (j1 + 1, QUAD, ew(j1 + 1), 2)
                cursors[g] += 2 * QUAD
            elif rem >= 2:
                ne = (rem + 1) // 2
                no = rem // 2
                out.append((j1, ne, ew(j1), 2))
                if no:
                    pend[g] = (j1 + 1, no, ew(j1 + 1), 2)
                cursors[g] += rem
            else:
                out.append((j1, 1, ew(j1), 1))
                cursors[g] += 1
        if not alive:
            break
    return out


def _relayout():
    """Recompute processing order / stage layout from current knobs."""
    global _QUADS, _POFF, P_PAD, _CHUNKS
    _QUADS = _quads()
    _POFF = {}
    pp = 0
    for j1s, nb, w, js in _QUADS:
        for c in range(nb):
            _POFF[j1s + c * js] = pp
            pp += w
    P_PAD = pp
    _CHUNKS = []
    acc = 0
    for j1s, nb, w, js in _QUADS:
        if acc + nb * w > CH_MAX:
            _CHUNKS.append(acc)
            acc = 0
        acc += nb * w
    if acc:
        _CHUNKS.append(acc)
    assert sum(_CHUNKS) == P_PAD


_relayout()


def ap2d(sliced, dims):
    """Copy of AP `sliced` with its free dims replaced by [step, count] pairs."""
    c = sliced.copy()
    v = c.ap
    part = [list(v[0])]
    while len(v) > 0:
        v.pop()
    for d in part + [list(x) for x in dims]:
        v.append(d)
    c.ap = v
    return c


A_EVERY = 0       # among B-region quads, every A_EVERY-th becomes class A


def _quad_class(qi, j1):
    if j1 >= J_SPLIT:
        # C-units (ACT evac + GpSimd mult) live in the A-region, where DVE
        # ops are PSUM-sourced and leave the shared SBUF port free
        if C_EVERY and qi % C_EVERY == C_EVERY - 1:
            return "C"
        return "A"
    if A_EVERY and qi % A_EVERY == A_EVERY - 1:
        return "A"
    return "B"


def _build_nc():
    nc = bacc.Bacc("TRN2", target_bir_lowering=False, debug=False,
                   num_devices=NCORES)

    xs_d = nc.dram_tensor("xs2", (2, BS, NF + XPAD), F16,
                          kind="ExternalInput").ap()
    xt_d = nc.dram_tensor("xt4", (4, K, 128 * 128), F16,
                          kind="ExternalInput").ap()
    wp_d = [nc.dram_tensor(f"wp{g}", (K, _GW[g] + _WPAD), F16,
                           kind="ExternalInput").ap() for g in range(4)]
    out_d = nc.dram_tensor("out", (BS * P_PAD,), F16, kind="ExternalOutput").ap()

    with TileContext(nc) as tc:
        with tc.tile_pool(name="sb", bufs=1) as sb, \
             tc.tile_pool(name="stg", bufs=STAGE_BUFS) as stg, \
             tc.tile_pool(name="tp", bufs=TMP_BUFS) as tp, \
             tc.tile_pool(name="ps", bufs=PSUM_BUFS, space="PSUM") as ps:

            xe = sb.tile([128, NF + XPAD], F16, tag="xe")
            xo = sb.tile([128, NF + XPAD], F16, tag="xo")
            nc.sync.dma_start(out=xe[:, :], in_=xs_d[0])
            nc.sync.dma_start(out=xo[:, :], in_=xs_d[1])

            xt = sb.tile([128, 128 * 128], F16, tag="xt")
            wp = sb.tile([128, _GW[0] + _WPAD], F16, tag="wp")
            for g in range(4):
                nc.sync.dma_start(out=xt[32 * g:32 * g + K, :], in_=xt_d[g])
                nc.sync.dma_start(out=wp[32 * g:32 * g + K, 0:_GW[g] + _WPAD],
                                  in_=wp_d[g][:])

            def lhs(j1):
                g = j1 // 128
                r = j1 - 128 * g
                return xt[32 * g:32 * g + K, r * 128:(r + 1) * 128]

            def rhs(j1, n):
                g = j1 // 128
                lo = _off(j1) - _GOFF[g]
                return wp[32 * g:32 * g + K, lo:lo + n]

            def xsl_a(col, cnt, step, n):
                # alignment-free slice (1x-mode consumers)
                return ap2d(xe[:, col:col + 1], [[step, cnt], [1, n]])

            def xsl_b(col, cnt, step, n):
                # 4B-aligned fp16 slice via parity copies (2x-mode consumers)
                t, c0 = (xe, col) if col % 2 == 0 else (xo, col - 1)
                return ap2d(t[:, c0:c0 + 1], [[step, cnt], [1, n]])

            if DMA_ONLY:
                zstage = sb.tile([128, CH_MAX], F16, tag="zstage")
                nc.vector.memset(zstage[:, :], 0.0)
                for _rep in range(REPS):
                    s = 0
                    for u in _CHUNKS:
                        dst = out_d[s * 128:(s + u) * 128]
                        dst = dst.rearrange("(p f) -> p f", p=128)
                        nc.sync.dma_start(out=dst, in_=zstage[:, 0:u])
                        s += u
            else:
                for _rep in range(REPS):
                    main_pass(nc, stg, tp, ps, xsl_a, xsl_b, lhs, rhs, out_d)

    nc.compile()
    return nc


def main_pass(nc, stg, tp, ps, xsl_a, xsl_b, lhs, rhs, out_d):
    used = 0
    chunk_base = 0
    stage = stg.tile([128, CH_MAX], F16, tag="stage")

    def flush():
        nonlocal used, chunk_base, stage
        if used == 0:
            return
        if not NO_DMA:
            dst = out_d[chunk_base * 128:(chunk_base + used) * 128]
            dst = dst.rearrange("(p f) -> p f", p=128)
            nc.sync.dma_start(out=dst, in_=stage[:, 0:used])
        chunk_base += used
        used = 0
        if chunk_base < P_PAD:
            stage = stg.tile([128, CH_MAX], F16, tag="stage")

    for qi, (j1s, nb, w, js) in enumerate(_QUADS):
        if used + nb * w > CH_MAX:
            flush()
        if DMA_ONLY:
            used += nb * w
            continue
        cls = _quad_class(qi, j1s)
        psum = ps.tile([128, 512 * QUAD], mybir.dt.float32, tag="psum")
        for c in range(nb):
            nc.tensor.matmul(psum[:, 512 * c:512 * c + w],
                             lhs(j1s + c * js), rhs(j1s + c * js, w),
                             start=True, stop=True,
                             tile_position=(32 * ((j1s + c * js) // 128), 0))
        O = used
        if SKIP_CONSUME:
            used += nb * w
            continue
        if cls == "A":
            nc.vector.tensor_mul(
                out=ap2d(stage[:, O:O + 1], [[w, nb], [1, w]]),
                in0=ap2d(psum[:, 0:1], [[512, nb], [1, w]]),
                in1=xsl_a(j1s + 1, nb, js, w))
        else:
            tmp = tp.tile([128, 512 * QUAD], F16, tag="tmp")
            nc.scalar.copy(
                out=ap2d(tmp[:, 0:1], [[512, nb], [1, w]]),
                in_=ap2d(psum[:, 0:1], [[512, nb], [1, w]]))
            if SKIP_MULT:
                used += nb * w
                continue
            if cls == "B" and js == 2:
                # same-parity pair: one 2x-mode op covers both blocks
                nc.vector.tensor_mul(
                    out=ap2d(stage[:, O:O + 1], [[w, nb], [1, w]]),
                    in0=ap2d(tmp[:, 0:1], [[512, nb], [1, w]]),
                    in1=xsl_b(j1s + 1, nb, 2, w))
            elif cls == "B":
                ne = (nb + 1) // 2   # even c: 0, 2
                no = nb // 2         # odd  c: 1, 3
                nc.vector.tensor_mul(
                    out=ap2d(stage[:, O:O + 1], [[2 * w, ne], [1, w]]),
                    in0=ap2d(tmp[:, 0:1], [[1024, ne], [1, w]]),
                    in1=xsl_b(j1s + 1, ne, 2, w))
                if no:
                    nc.vector.tensor_mul(
                        out=ap2d(stage[:, O + w:O + w + 1],
                                 [[2 * w, no], [1, w]]),
                        in0=ap2d(tmp[:, 512:513], [[1024, no], [1, w]]),
                        in1=xsl_b(j1s + 2, no, 2, w))
            else:  # C
                nc.gpsimd.tensor_mul(
                    out=ap2d(stage[:, O:O + 1], [[w, nb], [1, w]]),
                    in0=ap2d(tmp[:, 0:1], [[512, nb], [1, w]]),
                    in1=xsl_a(j1s + 1, nb, js, w))
        used += nb * w
    flush()


_NC_CACHE = None


def kernel(x, weight):
    global _NC_CACHE, LAST_RESULT, _last_in_maps
    x = np.ascontiguousarray(x, dtype=np.float32)
    weight = np.ascontiguousarray(weight, dtype=np.float32)
    assert x.shape == (B, NF) and weight.shape == (NF, K)

    # ---- host-side weight-derived constants (fp16)
    i1, i2 = np.triu_indices(NF, k=1)
    wp_full = weight[i2].T.astype(_f16np)                # [K, P] = w[j2, k]
    wp_in = {}
    for g in range(4):
        arr = np.zeros((K, _GW[g] + _WPAD), dtype=_f16np)
        arr[:, 0:_GW[g]] = wp_full[:, _GOFF[g]:_GOFF[g + 1]]
        wp_in[f"wp{g}"] = arr

    # ---- per-core inputs
    in_maps = []
    for c in range(NCORES):
        xc = x[c * BS:(c + 1) * BS]                      # [128, 512] f32
        xh = xc.astype(_f16np)
        xs2 = np.zeros((2, BS, NF + XPAD), dtype=_f16np)
        xs2[0, :, 0:NF] = xh
        xs2[1, :, 0:NF - 1] = xh[:, 1:]
        xct = xc.T                                        # [512, 128] f32
        xt4 = np.empty((4, K, 128 * 128), dtype=_f16np)
        for g in range(4):
            for k in range(K):
                # lhsT[k, r*128 + b] = x[b, j1] * w[j1, k], j1-major rows
                zk = (xct[128 * g:128 * (g + 1)]
                      * weight[128 * g:128 * (g + 1), k:k + 1]).astype(_f16np)
                xt4[g, k] = zk.reshape(-1)
        m = {"xs2": xs2, "xt4": xt4}
        m.update(wp_in)
        in_maps.append(m)

    _last_in_maps = in_maps
    if _NC_CACHE is None:
        _NC_CACHE = _build_nc()
    nc = _NC_CACHE

    res = bass_utils.run_bass_kernel_spmd(nc, in_maps,
                                          core_ids=list(range(NCORES)),
                                          trace=TRACE)
    LAST_RESULT = {"exec_time_ns": res.exec_time_ns,
                   "trace": res.instructions_and_trace}

    # ---- host unpack: de-chunk, de-pad, upcast
    idx = np.empty(P, dtype=np.int64)
    for j1, (po, n) in enumerate(
            (_POFF[j], NF - 1 - j) for j in range(NF - 1)):
        idx[_off(j1):_off(j1) + n] = np.arange(po, po + n)
    cores = []
    for r in res.results:
        flat = r["out"].reshape(-1)
        b2 = np.empty((BS, P_PAD), dtype=_f16np)
        s = 0
        for u in _CHUNKS:
            b2[:, s:s + u] = flat[128 * s:128 * (s + u)].reshape(BS, u)
            s += u
        cores.append(b2)
    blob_all = np.concatenate(cores, axis=0)              # [1024, P_PAD]
    out = blob_all[:, idx].astype(np.float32)
    return out
